# revision 1
# baseline (speedup 1.0000x reference)
"""GVSL loss (NCC + MSE + smoothness) as a distributed Bass kernel on 8 TRN2 cores.

Sharding: batch(2) x depth-quarters(4) = 8 shards. Each core computes partial
sums for its 32-deep output slab (with 4-voxel halo for the 9^3 box filter);
the final scalar reductions happen on the host.
"""

import os
import sys

for _p in ("/opt/trn_rl_repo",):
    if _p not in sys.path:
        sys.path.insert(0, _p)

import numpy as np
import ml_dtypes

BF16NP = ml_dtypes.bfloat16

import concourse.bass as bass
import concourse.tile as tile
from concourse import bacc, mybir
from concourse.bass_utils import run_bass_kernel_spmd

F32 = mybir.dt.float32
BF16 = mybir.dt.bfloat16
AF = mybir.ActivationFunctionType
ALU = mybir.AluOpType

HP = 128          # partitions (H axis)
W = 128
D_FULL = 128
DQ = 32           # output depths per core
D_IN = DQ + 8     # slab rows incl. halo
WPAD = 137        # 5 zeros | 128 data | 4 zeros
WOFF = 5
NCHUNK = 2
DC_OUT = DQ // NCHUNK          # 16
DC_IN = DC_OUT + 8             # 24
DCPAD = 26                     # 1 zero | 24 data | 1 zero
DPOFF = 1
FLOW_D = DQ + 1                # 33
WIN3 = 729.0

N_IN = D_IN * WPAD             # 5760
N_CHUNK_IN = DC_IN * WPAD      # 3456
N_CHUNK_HB = DC_IN * W         # 3072  (H-boxed compact, per chunk)
N_DPAD = W * DCPAD             # 4096
N_BOX = W * DC_OUT             # 2048
N_RECON = DQ * W               # 4096
N_FLOW_C = FLOW_D * W          # 4224


# acc_all columns
COL_CC0 = 0          # cc sums -> cols 0..7 (chunk0: 2 slices, chunk1: 4)
COL_MSE = 8
COL_DX = 9           # +c, W-axis diffs (3 channels)
COL_DZ = 12          # +c, D-axis diffs
COL_DY = 16          # +c*8+j, H-axis diffs per psum chunk
ACC_W = 40

_CACHE = {}


def _build_program():
    nc = bacc.Bacc("TRN2", target_bir_lowering=False, debug=False, num_devices=8)

    d_imgsA = nc.dram_tensor("imgsA", [HP, N_IN], F32, kind="ExternalInput").ap()
    d_warped = nc.dram_tensor("warped", [HP, N_IN], F32, kind="ExternalInput").ap()
    d_recon = nc.dram_tensor("recon", [HP, N_RECON], BF16, kind="ExternalInput").ap()
    d_mseA = nc.dram_tensor("mseA", [HP, N_RECON], BF16, kind="ExternalInput").ap()
    d_flow = nc.dram_tensor("flow", [HP, 3 * N_FLOW_C], BF16, kind="ExternalInput").ap()
    d_bandp = nc.dram_tensor("bandp", [HP, HP], F32, kind="ExternalInput").ap()
    d_bandn = nc.dram_tensor("bandn", [HP, HP], F32, kind="ExternalInput").ap()
    d_bidiag = nc.dram_tensor("bidiag", [HP, HP - 1], BF16, kind="ExternalInput").ap()
    d_out = nc.dram_tensor("out", [HP, ACC_W], F32, kind="ExternalOutput").ap()

    from contextlib import ExitStack

    with tile.TileContext(nc) as tc, ExitStack() as es:
        pp = es.enter_context(tc.tile_pool(name="persist", bufs=1))
        fp = es.enter_context(tc.tile_pool(name="flowp", bufs=1))
        fdp = es.enter_context(tc.tile_pool(name="diffp", bufs=1))
        rp = es.enter_context(tc.tile_pool(name="reconp", bufs=1))
        sip = es.enter_context(tc.tile_pool(name="srcI", bufs=1))
        prp = es.enter_context(tc.tile_pool(name="prodp", bufs=1))
        cup = es.enter_context(tc.tile_pool(name="cump", bufs=2))
        dpp = es.enter_context(tc.tile_pool(name="dpadp", bufs=2))
        bxp = es.enter_context(tc.tile_pool(name="boxp", bufs=1))
        scp = es.enter_context(tc.tile_pool(name="scrp", bufs=1))
        psp = es.enter_context(tc.tile_pool(name="psum", bufs=1, space="PSUM"))

        acc = pp.tile([HP, ACC_W], F32, tag="acc", name="acc")[:]
        eps_ap = pp.tile([HP, 1], F32, tag="epsc", name="epsc")[:]
        nc.gpsimd.memset(eps_ap, 1e-5)
        bandp = pp.tile([HP, HP], F32, tag="bandp", name="bandp")[:]
        bandn = pp.tile([HP, HP], F32, tag="bandn", name="bandn")[:]
        bidiag = pp.tile([HP, HP - 1], BF16, tag="bidiag", name="bidiag")[:]
        inJ = pp.tile([HP, N_IN], F32, tag="inJ", name="inJ")[:]
        inI = sip.tile([HP, N_IN], F32, tag="inI", name="inI")[:]

        # input DMAs: first-chunk slab rows first so the scans start early
        NJh = (DC_IN // 2) * WPAD
        NJ0 = DC_IN * WPAD
        nc.sync.dma_start(out=bandp, in_=d_bandp)
        nc.sync.dma_start(out=bandn, in_=d_bandn)
        nc.sync.dma_start(out=bidiag, in_=d_bidiag)
        nc.sync.dma_start(out=inJ[:, 0:NJh], in_=d_imgsA[:, 0:NJh])
        nc.sync.dma_start(out=inJ[:, NJh:NJ0], in_=d_imgsA[:, NJh:NJ0])
        nc.sync.dma_start(out=inI[:, 0:NJ0], in_=d_warped[:, 0:NJ0])
        nc.sync.dma_start(out=inJ[:, NJ0:], in_=d_imgsA[:, NJ0:])
        nc.sync.dma_start(out=inI[:, NJ0:], in_=d_warped[:, NJ0:])
        inJ_r = inJ.rearrange("p (d w) -> p d w", w=WPAD)
        inI_r = inI.rearrange("p (d w) -> p d w", w=WPAD)

        recon = rp.tile([HP, N_RECON], BF16, tag="recon", name="recon")[:]
        mseA = rp.tile([HP, N_RECON], BF16, tag="mseA", name="mseA")[:]
        nc.sync.dma_start(out=recon, in_=d_recon)
        nc.sync.dma_start(out=mseA, in_=d_mseA)
        d_flow_r = d_flow.rearrange("p (c d w) -> p c d w", c=3, w=W)

        def ncc_chunk(ch):
            r0 = ch * DC_OUT
            Jc2 = inJ_r[:, r0 : r0 + DC_IN, :].rearrange("p d w -> p (d w)")
            Ic2 = inI_r[:, r0 : r0 + DC_IN, :].rearrange("p d w -> p (d w)")

            boxes = {}
            for v in ("J", "I", "II", "JJ", "IJ"):
                if v == "J":
                    src2 = Jc2
                elif v == "I":
                    src2 = Ic2
                else:
                    prod = prp.tile([HP, N_CHUNK_IN], F32, tag="prod", name="prod")[:]
                    if v == "II":
                        nc.scalar.activation(prod, Ic2, AF.Square)
                    elif v == "JJ":
                        nc.scalar.activation(prod, Jc2, AF.Square)
                    else:
                        nc.vector.tensor_mul(prod, Ic2, Jc2)
                    src2 = prod

                # W-axis cumsum in two chained halves (box diff is fused
                # into the H-box matmuls via the +/- band pair)
                NH = N_CHUNK_IN // 2
                cum_a = cup.tile([HP, NH], F32, tag="cuma", name="cuma")[:]
                cum_b = cup.tile([HP, NH], F32, tag="cumb", name="cumb")[:]
                nc.vector.tensor_tensor_scan(
                    cum_a, src2[:, 0:NH], src2[:, 0:NH],
                    0.0, op0=ALU.add, op1=ALU.bypass,
                )
                nc.vector.tensor_tensor_scan(
                    cum_b, src2[:, NH:], src2[:, NH:],
                    cum_a[:, NH - 1 : NH], op0=ALU.add, op1=ALU.bypass,
                )
                cum_ar = cum_a.rearrange("p (d w) -> p d w", w=WPAD)
                cum_br = cum_b.rearrange("p (d w) -> p d w", w=WPAD)

                dpad = dpp.tile([HP, N_DPAD], F32, tag="dpad", name="dpad")[:]
                dpad_r = dpad.rearrange("p (w dp) -> p w dp", dp=DCPAD)
                nc.gpsimd.memset(dpad_r[:, :, 0:DPOFF], 0.0)
                nc.gpsimd.memset(dpad_r[:, :, DPOFF + DC_IN : DCPAD], 0.0)

                for j in range(N_CHUNK_HB // 512):
                    dlo = 4 * j
                    ps = psp.tile([HP, 512], F32, tag="ps", name="ps", bufs=6)[:]
                    cr = cum_ar if j < 3 else cum_br
                    dl = dlo if j < 3 else dlo - 12
                    rhs9 = cr[:, dl : dl + 4, 9 : 9 + W]
                    rhs0 = cr[:, dl : dl + 4, 0:W]
                    nc.tensor.matmul(ps, bandp, rhs9, start=True, stop=False)
                    nc.tensor.matmul(ps, bandn, rhs0, start=False, stop=True)
                    ps_wd = ps.rearrange("p (s w) -> p w s", w=W)
                    nc.scalar.copy(
                        dpad_r[:, :, DPOFF + dlo : DPOFF + dlo + 4], ps_wd
                    )

                # D-axis cumsum (in place) + diff -> final 9^3 box sums
                nc.vector.tensor_tensor_scan(
                    dpad, dpad, dpad, 0.0, op0=ALU.add, op1=ALU.bypass
                )
                cumd_r = dpad.rearrange("p (w dp) -> p w dp", dp=DCPAD)
                B = bxp.tile([HP, N_BOX], F32, tag=f"box{v}", name=f"box{v}")[:]
                B_r = B.rearrange("p (w d) -> p w d", d=DC_OUT)
                nc.vector.tensor_sub(
                    B_r,
                    cumd_r[:, :, 9 : 9 + DC_OUT],
                    cumd_r[:, :, 0 : 0 + DC_OUT],
                )
                boxes[v] = B

            # cc math in 2 slices so DVE and ACT pipeline across slices
            NS = N_BOX // 2
            for sl in range(2):
                lo, hi = sl * NS, (sl + 1) * NS
                BJ = boxes["J"][:, lo:hi]
                BI = boxes["I"][:, lo:hi]
                BII = boxes["II"][:, lo:hi]
                BJJ = boxes["JJ"][:, lo:hi]
                BIJ = boxes["IJ"][:, lo:hi]
                s1 = scp.tile([HP, NS], F32, tag="s1", name="s1")[:]
                s2 = scp.tile([HP, NS], F32, tag="s2", name="s2")[:]
                s3 = scp.tile([HP, NS], F32, tag="s3", name="s3")[:]

                nc.vector.tensor_mul(s1, BI, BJ)
                nc.vector.scalar_tensor_tensor(
                    s2, s1, -1.0 / WIN3, BIJ, op0=ALU.mult, op1=ALU.add
                )  # cross
                nc.scalar.activation(s1, s2, AF.Square)   # cross^2
                nc.scalar.activation(s2, s1, AF.Ln)       # ln(cross^2)
                nc.scalar.activation(s1, BI, AF.Square)
                nc.vector.scalar_tensor_tensor(
                    s3, s1, -1.0 / WIN3, BII, op0=ALU.mult, op1=ALU.add
                )  # I_var
                nc.scalar.activation(s1, BJ, AF.Square)
                nc.vector.scalar_tensor_tensor(
                    BII, s1, -1.0 / WIN3, BJJ, op0=ALU.mult, op1=ALU.add
                )  # J_var (overwrites dead BII)
                nc.vector.tensor_mul(s1, s3, BII)         # I_var * J_var
                nc.scalar.activation(s3, s1, AF.Ln, bias=eps_ap)
                nc.vector.tensor_sub(s1, s2, s3)
                col = COL_CC0 + ch * 2 + sl
                nc.scalar.activation(
                    s3, s1, AF.Exp, accum_out=acc[:, col : col + 1]
                )

        def flow_compute():
            # MSE
            mbuf = fdp.tile([HP, N_RECON], BF16, tag="dbuf", name="dbuf")[:]
            nc.vector.tensor_sub(mbuf, mseA, recon)
            nc.scalar.activation(
                mbuf, mbuf, AF.Square, accum_out=acc[:, COL_MSE : COL_MSE + 1]
            )
            for c in range(3):
                fc = fp.tile([HP, N_FLOW_C], BF16, tag="fc", name="fc", bufs=2)[:]
                nc.sync.dma_start(
                    out=fc, in_=d_flow_r[:, c].rearrange("p d w -> p (d w)")
                )
                fc_r = fc.rearrange("p (d w) -> p d w", w=W)

                # W-axis diffs (innermost)
                db = fdp.tile([HP, N_RECON], BF16, tag="dbuf", name="dbuf")[:]
                db_x = db.rearrange("p (d w) -> p d w", w=W)[:, :, 0 : W - 1]
                nc.vector.tensor_sub(
                    db_x, fc_r[:, 0:DQ, 1:W], fc_r[:, 0:DQ, 0 : W - 1]
                )
                col = COL_DX + c
                nc.scalar.activation(
                    db.rearrange("p (d w) -> p d w", w=W)[:, :, 0 : W - 1],
                    db.rearrange("p (d w) -> p d w", w=W)[:, :, 0 : W - 1],
                    AF.Square,
                    accum_out=acc[:, col : col + 1],
                )

                # H-axis diffs on the PE: psum = bidiag^T @ fc, squared in
                # place on PSUM (one acc column per psum chunk)
                fc_flat = fc_r[:, 0:DQ, :].rearrange("p d w -> p (d w)")
                for j in range(N_RECON // 512):
                    ps = psp.tile([HP, 512], F32, tag="fps", name="fps", bufs=2)[:]
                    nc.tensor.matmul(
                        ps[0 : HP - 1, :],
                        bidiag,
                        fc_flat[:, 512 * j : 512 * (j + 1)],
                        start=True,
                        stop=True,
                    )
                    col = COL_DY + c * 8 + j
                    nc.scalar.activation(
                        ps[0 : HP - 1, :],
                        ps[0 : HP - 1, :],
                        AF.Square,
                        accum_out=acc[0 : HP - 1, col : col + 1],
                    )

                # D-axis diffs
                db = fdp.tile([HP, N_RECON], BF16, tag="dbuf", name="dbuf")[:]
                nc.vector.tensor_sub(
                    db,
                    fc_r[:, 1 : DQ + 1, :].rearrange("p d w -> p (d w)"),
                    fc_r[:, 0:DQ, :].rearrange("p d w -> p (d w)"),
                )
                col = COL_DZ + c
                nc.scalar.activation(
                    db, db, AF.Square, accum_out=acc[:, col : col + 1]
                )

        ncc_chunk(0)
        ncc_chunk(1)
        flow_compute()
        nc.sync.dma_start(out=d_out, in_=acc)

    nc.compile()
    return nc


def _make_band() -> tuple[np.ndarray, np.ndarray, np.ndarray]:
    k = np.arange(HP)
    band = (np.abs(k[:, None] - k[None, :]) <= 4).astype(np.float32)
    m = np.arange(HP - 1)
    bidiag = np.zeros((HP, HP - 1), BF16NP)
    bidiag[m + 1, m] = 1.0
    bidiag[m, m] = -1.0
    return band, -band, bidiag


def _shard_inputs(imgsA, recon_A, warped_BA, flow_BA):
    bandp, bandn, bidiag = _make_band()
    in_maps = []
    for core in range(8):
        b, q = divmod(core, 4)
        d0 = DQ * q

        def slab(vol):
            s = np.zeros((HP, D_IN, WPAD), np.float32)
            lo, hi = d0 - 4, d0 + DQ + 4
            clo, chi = max(lo, 0), min(hi, D_FULL)
            s[:, clo - lo : chi - lo, WOFF : WOFF + W] = np.ascontiguousarray(
                vol[clo:chi].transpose(1, 0, 2)
            )
            return s.reshape(HP, N_IN)

        rec = np.ascontiguousarray(
            recon_A[b, 0, d0 : d0 + DQ].transpose(1, 0, 2)
        ).astype(BF16NP).reshape(HP, N_RECON)
        msea = np.ascontiguousarray(
            imgsA[b, 0, d0 : d0 + DQ].transpose(1, 0, 2)
        ).astype(BF16NP).reshape(HP, N_RECON)

        fl = np.empty((HP, 3, FLOW_D, W), BF16NP)
        hi = min(d0 + FLOW_D, D_FULL)
        n = hi - d0
        fl[:, :, :n] = flow_BA[b, :, d0:hi].transpose(2, 0, 1, 3)
        if n < FLOW_D:
            fl[:, :, n:] = fl[:, :, n - 1 : n]

        in_maps.append(
            {
                "imgsA": slab(imgsA[b, 0]),
                "warped": slab(warped_BA[b, 0]),
                "recon": rec,
                "mseA": msea,
                "flow": np.ascontiguousarray(fl).reshape(HP, 3 * N_FLOW_C),
                "bandp": bandp,
                "bandn": bandn,
                "bidiag": bidiag,
            }
        )
    return in_maps


def _install_profile_shim():
    """Wire up NTFF profiling under axon when antenv.axon_hooks is absent."""
    try:
        import antenv.axon_hooks  # noqa: F401

        return True
    except ImportError:
        pass
    import contextlib
    import ctypes
    import types

    so_path = "/opt/axon/libaxon_pjrt.so"
    if not os.path.exists(so_path):
        return False
    lib = ctypes.CDLL(so_path)
    if not hasattr(lib, "axon_start_nrt_profile"):
        return False
    lib.axon_start_nrt_profile.argtypes = [
        ctypes.POINTER(ctypes.c_int64),
        ctypes.c_size_t,
    ]
    lib.axon_start_nrt_profile.restype = ctypes.c_int64
    lib.axon_stop_nrt_profile.argtypes = [ctypes.c_char_p]
    lib.axon_stop_nrt_profile.restype = ctypes.c_int64

    @contextlib.contextmanager
    def _hook(output_dir, device_ids):
        import jax

        jax.devices()
        if device_ids:
            ids = (ctypes.c_int64 * len(device_ids))(*device_ids)
            rc = lib.axon_start_nrt_profile(ids, len(device_ids))
        else:
            rc = lib.axon_start_nrt_profile(None, 0)
        if rc != 0:
            raise RuntimeError(f"axon_start_nrt_profile rc={rc}")
        try:
            yield
        finally:
            n = lib.axon_stop_nrt_profile(str(output_dir).encode())
            print(f"ntff profile: {n} file(s) written to {output_dir}")

    mod = types.ModuleType("antenv.axon_hooks")
    mod.get_axon_ntff_profile_hook = lambda: _hook
    mod.set_axon_ntff_profile_hook = lambda h: None
    import antenv

    sys.modules["antenv.axon_hooks"] = mod
    antenv.axon_hooks = mod

    # keep profile artifacts local instead of uploading to fishnet
    import concourse.bass_utils as _bu

    _bu.upload_artifacts = lambda tmpdir: tmpdir
    return True


LAST_EXEC_NS = None
LAST_RESULTS = None


def kernel(imgsA, recon_A, warped_BA, flow_BA):
    global LAST_EXEC_NS, LAST_RESULTS
    if "nc" not in _CACHE:
        _CACHE["nc"] = _build_program()
    nc = _CACHE["nc"]

    in_maps = _shard_inputs(
        np.asarray(imgsA, np.float32),
        np.asarray(recon_A, np.float32),
        np.asarray(warped_BA, np.float32),
        np.asarray(flow_BA, np.float32),
    )
    trace = os.environ.get("GVSL_TRACE", "0") == "1"
    if trace:
        trace = _install_profile_shim()
    tmpdir = os.environ.get("GVSL_TRACE_DIR") or None
    res = run_bass_kernel_spmd(
        nc, in_maps, core_ids=list(range(8)), trace=trace, tmpdir=tmpdir
    )
    LAST_EXEC_NS = res.exec_time_ns
    LAST_RESULTS = res

    cc = mse = dx = dy = dz = 0.0
    for r in res.results:
        o = np.asarray(r["out"], np.float64)
        cc += o[:, COL_CC0 : COL_CC0 + 4].sum()
        mse += o[:, COL_MSE].sum()
        dx += o[:, COL_DX : COL_DX + 3].sum()
        dy += o[: HP - 1, COL_DY : COL_DY + 24].sum()
        dz += o[:, COL_DZ : COL_DZ + 3].sum()

    n_vox = 2 * 1 * 128 * 128 * 128
    n_d = 2 * 3 * 127 * 128 * 128
    ncc_loss = 1.0 - cc / n_vox
    mse_loss = mse / n_vox
    smooth_loss = (dx / n_d + dy / n_d + dz / n_d) / 3.0
    return (
        np.float32(ncc_loss),
        np.float32(mse_loss),
        np.float32(smooth_loss),
    )



# revision 5
# speedup vs baseline: 1.5948x; 1.5948x over previous
"""GVSL loss (NCC + MSE + smoothness) as a distributed Bass kernel on 8 TRN2 cores.

Sharding: batch(2) x depth-quarters(4) = 8 shards; each core owns a 32-deep
output slab (40 input rows with the 4-voxel halo).

NCC pipeline per volume (J, I, II, JJ, IJ), fp16 throughout (fp16's 11-bit
mantissa keeps the cross-term cancellation error ~0.1% of signal):
  - W-axis 9-box via ONE DVE window-scan (op0=add, op1=subtract on a
    9-shifted view of the same padded tile)
  - H-axis 9-box + D-axis 9-box fused on the PE: 9 d-shifted fp16 band
    matmuls accumulated into one PSUM group
  - PSUM -> SBUF fp16 staging (scale 1/27 for S_I, S_J), cc math on
    [128, 4096] fp16 tiles, Ln/Exp trick, accumulated into acc columns.
"""

import os
import sys

for _p in ("/opt/trn_rl_repo",):
    if _p not in sys.path:
        sys.path.insert(0, _p)

import numpy as np
import ml_dtypes

BF16NP = ml_dtypes.bfloat16

import concourse.bass as bass
import concourse.tile as tile
from concourse import bacc, mybir
from concourse.bass_utils import run_bass_kernel_spmd

F32 = mybir.dt.float32
BF16 = mybir.dt.bfloat16
F16 = mybir.dt.float16
AF = mybir.ActivationFunctionType
ALU = mybir.AluOpType

HP = 128          # partitions (H axis)
W = 128
D_FULL = 128
DQ = 32           # output depths per core
DSLAB = 40        # input slab rows incl. +-4 halo
WPAD = 137        # 9 zeros | 128 data
LEAD = 9          # leading zeros before row 0 (for the shifted scan view)
TAIL = 4          # trailing zeros after the last row
NP_ = LEAD + DSLAB * WPAD + TAIL   # 5493 padded input length
NO = DSLAB * WPAD + TAIL           # 5484 valid scan-output length
NOA = NP_                          # alloc size for scan output views
NSTG = DQ * W                      # 4096 staged box elems
FLOW_D = DQ + 1    # 33
N_RECON = DQ * W   # 4096
N_FLOW_C = FLOW_D * W  # 4224

# acc columns
COL_CC = 0
COL_MSE = 1
COL_DX = 2         # +c (3 channels)
COL_DZ = 5         # +c
COL_DY = 8         # +c*8+j per psum chunk
ACC_W = 32

_CACHE = {}


def _build_program():
    nc = bacc.Bacc("TRN2", target_bir_lowering=False, debug=False, num_devices=8)

    d_J = nc.dram_tensor("J", [HP, NP_], F16, kind="ExternalInput").ap()
    d_I = nc.dram_tensor("I", [HP, NP_], F16, kind="ExternalInput").ap()
    d_recon = nc.dram_tensor("recon", [HP, N_RECON], BF16, kind="ExternalInput").ap()
    d_mseA = nc.dram_tensor("mseA", [HP, N_RECON], BF16, kind="ExternalInput").ap()
    d_flow = nc.dram_tensor("flow", [HP, 3 * N_FLOW_C], BF16, kind="ExternalInput").ap()
    d_band = nc.dram_tensor("band", [HP, HP], F16, kind="ExternalInput").ap()
    d_bidiag = nc.dram_tensor("bidiag", [HP, HP - 1], BF16, kind="ExternalInput").ap()
    d_out = nc.dram_tensor("out", [HP, ACC_W], F32, kind="ExternalOutput").ap()

    from contextlib import ExitStack

    with tile.TileContext(nc) as tc, ExitStack() as es:
        pp = es.enter_context(tc.tile_pool(name="persist", bufs=1))
        prp = es.enter_context(tc.tile_pool(name="prodp", bufs=2))
        op = es.enter_context(tc.tile_pool(name="outp", bufs=2))
        stp = es.enter_context(tc.tile_pool(name="stagep", bufs=1))
        scp = es.enter_context(tc.tile_pool(name="scrp", bufs=1))
        fp = es.enter_context(tc.tile_pool(name="flowp", bufs=1))
        fdp = es.enter_context(tc.tile_pool(name="diffp", bufs=1))
        rp = es.enter_context(tc.tile_pool(name="reconp", bufs=1))
        psp = es.enter_context(tc.tile_pool(name="psum", bufs=1, space="PSUM"))
        fpp = es.enter_context(tc.tile_pool(name="fpsum", bufs=1, space="PSUM"))

        acc = pp.tile([HP, ACC_W], F32, tag="acc", name="acc")[:]
        band = pp.tile([HP, HP], F16, tag="band", name="band")[:]
        bidiag = pp.tile([HP, HP - 1], BF16, tag="bidiag", name="bidiag")[:]
        PJ = pp.tile([HP, NP_], F16, tag="PJ", name="PJ")[:]
        PI = pp.tile([HP, NP_], F16, tag="PI", name="PI")[:]

        nc.sync.dma_start(out=band, in_=d_band)
        nc.sync.dma_start(out=bidiag, in_=d_bidiag)
        NJh = NP_ // 2
        nc.sync.dma_start(out=PJ[:, 0:NJh], in_=d_J[:, 0:NJh])
        nc.sync.dma_start(out=PJ[:, NJh:], in_=d_J[:, NJh:])
        nc.sync.dma_start(out=PI[:, 0:NJh], in_=d_I[:, 0:NJh])
        nc.sync.dma_start(out=PI[:, NJh:], in_=d_I[:, NJh:])

        recon = rp.tile([HP, N_RECON], BF16, tag="recon", name="recon")[:]
        mseA = rp.tile([HP, N_RECON], BF16, tag="mseA", name="mseA")[:]
        nc.sync.dma_start(out=recon, in_=d_recon)
        nc.sync.dma_start(out=mseA, in_=d_mseA)
        d_flow_r = d_flow.rearrange("p (c d w) -> p c d w", c=3, w=W)

        # data region view helper: [p, DSLAB, 128] inside a padded tile
        def data_view(t):
            rows = t[:, LEAD : LEAD + DSLAB * WPAD].rearrange(
                "p (d w) -> p d w", w=WPAD
            )
            return rows[:, :, LEAD : LEAD + W]

        # pre-zero the two product buffers (pads stay zero afterwards)
        pbufs = []
        for i in range(2):
            pb = prp.tile([HP, NP_], F16, tag="prod", name=f"pz{i}")[:]
            nc.gpsimd.memset(pb, 0.0)
            pbufs.append(pb)

        # staging tiles (box sums, fp16; J/I scaled by 1/27)
        stage = {
            v: stp.tile([HP, NSTG], F16, tag=f"st{v}", name=f"st{v}")[:]
            for v in ("J", "I", "II", "JJ", "IJ")
        }

        def ncc_volume(v):
            if v == "J":
                P = PJ
            elif v == "I":
                P = PI
            else:
                P = prp.tile([HP, NP_], F16, tag="prod", name="prod")[:]
                pv = data_view(P)
                if v == "II":
                    nc.scalar.activation(pv, data_view(PI), AF.Square)
                elif v == "JJ":
                    nc.scalar.activation(pv, data_view(PJ), AF.Square)
                else:
                    nc.vector.tensor_mul(pv, data_view(PI), data_view(PJ))

            # W-axis 9-window sums via a single scan pass (two chained halves)
            O = op.tile([HP, NOA], F16, tag="obox", name="obox")[:]
            NH = (DSLAB // 2) * WPAD  # 2740, row-20 boundary
            nc.vector.tensor_tensor_scan(
                O[:, 0:NH], P[:, LEAD : LEAD + NH], P[:, 0:NH],
                0.0, op0=ALU.add, op1=ALU.subtract,
            )
            nc.vector.tensor_tensor_scan(
                O[:, NH:NO], P[:, LEAD + NH : NP_], P[:, NH:NO],
                O[:, NH - 1 : NH], op0=ALU.add, op1=ALU.subtract,
            )

            # H-box + D-box on PE: 9 d-shifted band matmuls into one psum group
            # shift-j view: rows r=j.. at 137-stride, out col 13+137r+w
            Vs = [
                O[:, 13 + WPAD * j : 13 + WPAD * j + WPAD * DQ].rearrange(
                    "p (d w) -> p d w", w=WPAD
                )
                for j in range(9)
            ]
            sc = 1.0 / 27.0 if v in ("J", "I") else 1.0
            for g in range(4):  # 8 output depths per group, 2 psum halves
                ps = psp.tile([HP, 1024], F32, tag="ps", name="ps", bufs=3)[:]
                for h in range(2):
                    r0 = 8 * g + 4 * h
                    for j in range(9):
                        nc.tensor.matmul(
                            ps[:, 512 * h : 512 * (h + 1)],
                            band,
                            Vs[j][:, r0 : r0 + 4, 0:W],
                            start=(j == 0),
                            stop=(j == 8),
                        )
                nc.scalar.activation(
                    stage[v][:, 1024 * g : 1024 * (g + 1)], ps,
                    AF.Copy, scale=sc,
                )

        def cc_math():
            eps_ap = pp.tile([HP, 1], F32, tag="epsc", name="epsc")[:]
            nc.gpsimd.memset(eps_ap, 1e-5)
            aJ, aI = stage["J"], stage["I"]
            s1 = scp.tile([HP, NSTG], F16, tag="s1", name="s1")[:]
            s2 = scp.tile([HP, NSTG], F16, tag="s2", name="s2")[:]
            s3 = scp.tile([HP, NSTG], F16, tag="s3", name="s3")[:]
            s4 = scp.tile([HP, NSTG], F16, tag="s4", name="s4")[:]

            nc.vector.tensor_mul(s1, aI, aJ)                  # a_I*a_J
            nc.vector.tensor_sub(s2, stage["IJ"], s1)         # cross
            nc.vector.tensor_mul(s1, s2, s2)                  # cross^2
            nc.scalar.activation(s2, s1, AF.Ln)               # ln(cross^2)
            nc.scalar.activation(s3, aI, AF.Square)           # a_I^2
            nc.vector.tensor_sub(s1, stage["II"], s3)         # I_var
            nc.scalar.activation(s3, aJ, AF.Square)
            nc.vector.tensor_sub(s4, stage["JJ"], s3)         # J_var
            nc.vector.tensor_mul(s1, s1, s4)                  # Ivar*Jvar
            nc.scalar.activation(s3, s1, AF.Ln, bias=eps_ap)  # ln(den+eps)
            nc.vector.tensor_sub(s1, s2, s3)
            nc.scalar.activation(
                s2, s1, AF.Exp, accum_out=acc[:, COL_CC : COL_CC + 1]
            )

        def flow_compute():
            # MSE
            mbuf = fdp.tile([HP, N_RECON], BF16, tag="dbuf", name="dbuf")[:]
            nc.vector.tensor_sub(mbuf, mseA, recon)
            nc.scalar.activation(
                mbuf, mbuf, AF.Square, accum_out=acc[:, COL_MSE : COL_MSE + 1]
            )
            for c in range(3):
                fc = fp.tile([HP, N_FLOW_C], BF16, tag="fc", name="fc", bufs=2)[:]
                nc.sync.dma_start(
                    out=fc, in_=d_flow_r[:, c].rearrange("p d w -> p (d w)")
                )
                fc_r = fc.rearrange("p (d w) -> p d w", w=W)

                # W-axis diffs (innermost)
                db = fdp.tile([HP, N_RECON], BF16, tag="dbuf", name="dbuf")[:]
                db_x = db.rearrange("p (d w) -> p d w", w=W)[:, :, 0 : W - 1]
                nc.vector.tensor_sub(
                    db_x, fc_r[:, 0:DQ, 1:W], fc_r[:, 0:DQ, 0 : W - 1]
                )
                col = COL_DX + c
                nc.scalar.activation(
                    db.rearrange("p (d w) -> p d w", w=W)[:, :, 0 : W - 1],
                    db.rearrange("p (d w) -> p d w", w=W)[:, :, 0 : W - 1],
                    AF.Square,
                    accum_out=acc[:, col : col + 1],
                )

                # H-axis diffs on the PE: psum = bidiag^T @ fc, squared on PSUM
                fc_flat = fc_r[:, 0:DQ, :].rearrange("p d w -> p (d w)")
                for j in range(N_RECON // 512):
                    ps = fpp.tile([HP, 512], F32, tag="fps", name="fps", bufs=2)[:]
                    nc.tensor.matmul(
                        ps[0 : HP - 1, :],
                        bidiag,
                        fc_flat[:, 512 * j : 512 * (j + 1)],
                        start=True,
                        stop=True,
                    )
                    col = COL_DY + c * 8 + j
                    nc.scalar.activation(
                        ps[0 : HP - 1, :],
                        ps[0 : HP - 1, :],
                        AF.Square,
                        accum_out=acc[0 : HP - 1, col : col + 1],
                    )

                # D-axis diffs
                db = fdp.tile([HP, N_RECON], BF16, tag="dbuf", name="dbuf")[:]
                nc.vector.tensor_sub(
                    db,
                    fc_r[:, 1 : DQ + 1, :].rearrange("p d w -> p (d w)"),
                    fc_r[:, 0:DQ, :].rearrange("p d w -> p (d w)"),
                )
                col = COL_DZ + c
                nc.scalar.activation(
                    db, db, AF.Square, accum_out=acc[:, col : col + 1]
                )

        for v in ("J", "I", "II", "JJ", "IJ"):
            ncc_volume(v)
        cc_math()
        flow_compute()
        nc.sync.dma_start(out=d_out, in_=acc)

    nc.compile()
    return nc


def _make_consts() -> tuple[np.ndarray, np.ndarray]:
    k = np.arange(HP)
    band = (np.abs(k[:, None] - k[None, :]) <= 4).astype(np.float16)
    m = np.arange(HP - 1)
    bidiag = np.zeros((HP, HP - 1), BF16NP)
    bidiag[m + 1, m] = 1.0
    bidiag[m, m] = -1.0
    return band, bidiag


def _shard_inputs(imgsA, recon_A, warped_BA, flow_BA):
    band, bidiag = _make_consts()
    in_maps = []
    for core in range(8):
        b, q = divmod(core, 4)
        d0 = DQ * q

        def slab(vol):
            # padded fp16 slab: LEAD zeros, DSLAB rows of [9 zeros|128 data],
            # TAIL zeros; volume centered by -0.5
            s = np.zeros((HP, NP_), np.float16)
            rows = s[:, LEAD : LEAD + DSLAB * WPAD].reshape(HP, DSLAB, WPAD)
            lo, hi = d0 - 4, d0 + DQ + 4
            clo, chi = max(lo, 0), min(hi, D_FULL)
            rows[:, clo - lo : chi - lo, LEAD : LEAD + W] = np.ascontiguousarray(
                vol[clo:chi].transpose(1, 0, 2)
            ).astype(np.float16)
            return s

        rec = np.ascontiguousarray(
            recon_A[b, 0, d0 : d0 + DQ].transpose(1, 0, 2)
        ).astype(BF16NP).reshape(HP, N_RECON)
        msea = np.ascontiguousarray(
            imgsA[b, 0, d0 : d0 + DQ].transpose(1, 0, 2)
        ).astype(BF16NP).reshape(HP, N_RECON)

        fl = np.empty((HP, 3, FLOW_D, W), BF16NP)
        hi = min(d0 + FLOW_D, D_FULL)
        n = hi - d0
        fl[:, :, :n] = flow_BA[b, :, d0:hi].transpose(2, 0, 1, 3)
        if n < FLOW_D:
            fl[:, :, n:] = fl[:, :, n - 1 : n]

        in_maps.append(
            {
                "J": slab(imgsA[b, 0]),
                "I": slab(warped_BA[b, 0]),
                "recon": rec,
                "mseA": msea,
                "flow": np.ascontiguousarray(fl).reshape(HP, 3 * N_FLOW_C),
                "band": band,
                "bidiag": bidiag,
            }
        )
    return in_maps


def _install_profile_shim():
    """Wire up NTFF profiling under axon when antenv.axon_hooks is absent."""
    try:
        import antenv.axon_hooks  # noqa: F401

        return True
    except ImportError:
        pass
    import contextlib
    import ctypes
    import types

    so_path = "/opt/axon/libaxon_pjrt.so"
    if not os.path.exists(so_path):
        return False
    lib = ctypes.CDLL(so_path)
    if not hasattr(lib, "axon_start_nrt_profile"):
        return False
    lib.axon_start_nrt_profile.argtypes = [
        ctypes.POINTER(ctypes.c_int64),
        ctypes.c_size_t,
    ]
    lib.axon_start_nrt_profile.restype = ctypes.c_int64
    lib.axon_stop_nrt_profile.argtypes = [ctypes.c_char_p]
    lib.axon_stop_nrt_profile.restype = ctypes.c_int64

    @contextlib.contextmanager
    def _hook(output_dir, device_ids):
        import jax

        jax.devices()
        if device_ids:
            ids = (ctypes.c_int64 * len(device_ids))(*device_ids)
            rc = lib.axon_start_nrt_profile(ids, len(device_ids))
        else:
            rc = lib.axon_start_nrt_profile(None, 0)
        if rc != 0:
            raise RuntimeError(f"axon_start_nrt_profile rc={rc}")
        try:
            yield
        finally:
            n = lib.axon_stop_nrt_profile(str(output_dir).encode())
            print(f"ntff profile: {n} file(s) written to {output_dir}")

    mod = types.ModuleType("antenv.axon_hooks")
    mod.get_axon_ntff_profile_hook = lambda: _hook
    mod.set_axon_ntff_profile_hook = lambda h: None
    import antenv

    sys.modules["antenv.axon_hooks"] = mod
    antenv.axon_hooks = mod

    # keep profile artifacts local instead of uploading to fishnet
    import concourse.bass_utils as _bu

    _bu.upload_artifacts = lambda tmpdir: tmpdir
    return True


LAST_EXEC_NS = None
LAST_RESULTS = None


def kernel(imgsA, recon_A, warped_BA, flow_BA):
    global LAST_EXEC_NS, LAST_RESULTS
    if "nc" not in _CACHE:
        _CACHE["nc"] = _build_program()
    nc = _CACHE["nc"]

    in_maps = _shard_inputs(
        np.asarray(imgsA, np.float32),
        np.asarray(recon_A, np.float32),
        np.asarray(warped_BA, np.float32),
        np.asarray(flow_BA, np.float32),
    )
    trace = os.environ.get("GVSL_TRACE", "0") == "1"
    if trace:
        trace = _install_profile_shim()
    tmpdir = os.environ.get("GVSL_TRACE_DIR") or None
    res = run_bass_kernel_spmd(
        nc, in_maps, core_ids=list(range(8)), trace=trace, tmpdir=tmpdir
    )
    LAST_EXEC_NS = res.exec_time_ns
    LAST_RESULTS = res

    cc = mse = dx = dy = dz = 0.0
    for r in res.results:
        o = np.asarray(r["out"], np.float64)
        cc += o[:, COL_CC].sum()
        mse += o[:, COL_MSE].sum()
        dx += o[:, COL_DX : COL_DX + 3].sum()
        dy += o[: HP - 1, COL_DY : COL_DY + 24].sum()
        dz += o[:, COL_DZ : COL_DZ + 3].sum()

    n_vox = 2 * 1 * 128 * 128 * 128
    n_d = 2 * 3 * 127 * 128 * 128
    ncc_loss = 1.0 - cc / n_vox
    mse_loss = mse / n_vox
    smooth_loss = (dx / n_d + dy / n_d + dz / n_d) / 3.0
    return (
        np.float32(ncc_loss),
        np.float32(mse_loss),
        np.float32(smooth_loss),
    )
